# revision 94
# baseline (speedup 1.0000x reference)
"""Trainium2 Bass kernel for a Linformer transformer block (nn_Block).

Shapes (hardcoded): B=2, N=8192, C=768, H=12, D=64, K=256, HID=3072.
Sharding: 8 cores, data-parallel over tokens (2048 tokens/core, batch-major:
cores 0-3 hold batch 0, cores 4-7 batch 1).

Algorithm (per core, T=2048 tokens):
  - LN1 apply emits h1 directly as x16-scaled fp8 in "chunk-paired" layout
    (DoubleRow pair chunks in adjacent bytes), so one DMA transpose of the
    fake-fp16 view yields the q-matmul rhs and the same tile feeds the
    ek/ev DoubleRow matmuls -- no fp16 h1 and no conversion passes.
  - Linformer reorder: ekhT = h1^T Ek (C x K partials over local tokens),
    AllReduce over the batch group (all four q^T groups overlap it), then
    k_projT = Wk^T ekhT and v_proj = evhT^T Wv -- eliminates the two big
    [T,C]x[C,C] GEMMs.
  - All weight-stationary GEMMs (q/k/v proj, out-proj, fc1, fc2) run in
    fp8e4m3 DoubleRow mode (0.5 cyc/row, 256-deep contraction): weights are
    hi+lo fp8 planes quantized on the host (W ~= Wh + Wl, accurate to
    ~2^-8), activations are a single fp8 plane emitted directly in fp8 by
    the producing instruction.
  - Attention per 512-token group: logits fp16, exp -> eT in fp8 straight
    from the ACT engine; o^T for a head pair accumulates all four DoubleRow
    matmuls (hi/lo v planes, zero-padded by head parity) into one PSUM
    tile, so one reciprocal + one scale op normalizes both heads.
  - LN2 rstd computed on DVE (Quake rsqrt + 2 Newton steps) to keep the
    ACT table stream free of Sqrt; stage 2 issues all exp groups before
    all gelu groups so the ACT table loads only twice.
  - Engine balance: softmax denominator sums split DVE/Pool, LN applies on
    Pool, PSUM-reading drains on DVE; stage 1 is DMA-bound (x in quad
    DMAs, Ek|Ev merged fp8, out in fp16) with the fc weights streaming
    during the AllReduce and attention windows.
"""

import sys
sys.path.insert(0, "/opt/trn_rl_repo")

import numpy as np
import ml_dtypes

import concourse.bass as bass
import concourse.mybir as mybir
import concourse.tile as tile
from concourse import bacc
from concourse.bass_utils import run_bass_kernel_spmd

F32 = mybir.dt.float32
F16 = mybir.dt.float16
I32 = mybir.dt.int32
F8 = mybir.dt.float8e4
AF = mybir.ActivationFunctionType
ALU = mybir.AluOpType
DR = mybir.MatmulPerfMode.DoubleRow

B, N, C = 2, 8192, 768
H, K = 12, 256
D = C // H                 # 64
HID = 4 * C                # 3072
EPS = 1e-6
NCORES = 8
T = (B * N) // NCORES      # 2048 tokens per core
NT = T // 128              # 16 token tiles
NG = T // 512              # 4 token groups
CK = C // 128              # 6 chunks of C
CP = CK // 2               # 3 DoubleRow pairs of C
HC = HID // 128            # 24 hidden chunks
HP = HC // 2               # 12 DoubleRow pairs of HID
KC = K // 128              # 2 K chunks
SCALE = float(D) ** -0.5   # 0.125
SX = 16.0                  # fp8 scale for h2 (LN2 output)
SW = 64.0                  # fp8 scale for fc1/fc2 weights
SH = 16.0                  # fp8 scale for h1T (q input)
SQW = 1024.0               # fp8 scale for qkv_w planes
SPW = 1024.0               # fp8 scale for proj_w planes
SEH = 8.0                  # fp8 scale for ekh/evh (AllReduce output)
SEK = 1024.0               # fp8 scale for Ek/Ev host planes
S1H = 16.0                 # fp8 scale for h1 (stage-1 ek/ev matmuls)
SV = 16.0                  # fp8 scale for v_proj
SO = 16.0                  # fp8 scale for oT (attention output)
RSQRT_MAGIC = 0x5F3759DF

_CACHE = {}


def _ln_stats(nc, pool, xt, tag):
    """LayerNorm stats for a (128, C) fp32 tile -> (rstd, -mu*rstd) (128,1).
    Stage-1 variant: ACT Sqrt + DVE reciprocal (no exp/gelu nearby)."""
    NSUB = 3  # 768 = 3 x 256 (BN_STATS_FMAX=512, gcd=256)
    stats = pool.tile([128, NSUB, 6], F32, tag=f"{tag}_stats")
    xv = xt.rearrange("p (j s) -> p j s", j=NSUB)
    for j in range(NSUB):
        nc.vector.bn_stats(stats[:, j, :], xv[:, j, :])
    mv = pool.tile([128, 2], F32, tag=f"{tag}_mv")
    nc.vector.bn_aggr(mv[:], stats[:])
    var = pool.tile([128, 1], F32, tag=f"{tag}_var")
    nc.vector.tensor_scalar_add(var[:], mv[:, 1:2], EPS)
    std = pool.tile([128, 1], F32, tag=f"{tag}_std")
    nc.scalar.activation(std[:], var[:], AF.Sqrt)
    rstd = pool.tile([128, 1], F32, tag=f"{tag}_rstd")
    nc.vector.reciprocal(rstd[:], std[:])
    nmr = pool.tile([128, 1], F32, tag=f"{tag}_nmr")
    nc.vector.scalar_tensor_tensor(nmr[:], mv[:, 0:1], -1.0, rstd[:],
                                   op0=ALU.mult, op1=ALU.mult)
    return rstd, nmr


def _rsqrt_dve(nc, pool, var_ap, width, tag):
    """rstd = var ** -0.5 on DVE only (Quake seed + 2 Newton steps).
    var_ap: fp32 [128, width] AP. Keeps Sqrt off the ACT table stream."""
    yb = pool.tile([128, width], I32, tag=f"{tag}_yb")
    nc.vector.tensor_scalar(yb[:], var_ap.bitcast(I32), 1, -1,
                            op0=ALU.logical_shift_right, op1=ALU.bitwise_xor)
    nc.vector.tensor_scalar(yb[:], yb[:], RSQRT_MAGIC + 1, None, op0=ALU.add)
    y = yb[:].bitcast(F32)
    h = pool.tile([128, width], F32, tag=f"{tag}_h")
    for _ in range(2):
        nc.vector.tensor_mul(h[:], y, y)
        nc.vector.scalar_tensor_tensor(h[:], var_ap, -0.5, h[:],
                                       op0=ALU.mult, op1=ALU.mult)
        nc.vector.tensor_scalar_add(h[:], h[:], 1.5)
        nc.vector.tensor_mul(y, y, h[:])
    return y


def build(ln1_triv, ln2_triv, kvb_zero, projb_zero, fc2b_zero):
    nc = bacc.Bacc("TRN2", target_bir_lowering=False, debug=False,
                   enable_asserts=True, num_devices=NCORES)

    x_s = nc.dram_tensor("x_s", [T, C], F16, kind="ExternalInput").ap()
    # qkv weights: [hi/lo, 128, pair, 2, 3C] fp8, [p,j,s,m] = W[(2j+s)*128+p, m]
    qkv_w8 = nc.dram_tensor("qkv_w8", [2, 128, CP, 2, 3 * C], F8,
                            kind="ExternalInput").ap()
    qkv_b = nc.dram_tensor("qkv_b", [3 * C], F32, kind="ExternalInput").ap()
    # Ek/Ev merged, single-plane fp8 (scaled x1024 on host): halves their
    # DMA bytes and keeps 512B contiguous runs (no sub-512B DMA penalty).
    ekev_s = nc.dram_tensor("ekev_s", [T, 2 * K], F8,
                            kind="ExternalInput").ap()
    proj_w8 = nc.dram_tensor("proj_w8", [2, 128, CP, 2, C], F8,
                             kind="ExternalInput").ap()
    proj_b = nc.dram_tensor("proj_b", [C], F32, kind="ExternalInput").ap()
    # fc1 weights: [hi/lo, 128, pair, 2, HID] fp8, [p,j,s,m] = W1[(2j+s)*128+p, m]
    fc1_w8 = nc.dram_tensor("fc1_w8", [2, 128, CP, 2, HID], F8,
                            kind="ExternalInput").ap()
    fc1_b = nc.dram_tensor("fc1_b", [HID], F32, kind="ExternalInput").ap()
    # fc2 weights: [hi/lo, 128, pair, 2, C] fp8, [p,j,s,c] = W2[(2j+s)*128+p, c]
    fc2_w8 = nc.dram_tensor("fc2_w8", [2, 128, HP, 2, C], F8,
                            kind="ExternalInput").ap()
    fc2_b = nc.dram_tensor("fc2_b", [C], F32, kind="ExternalInput").ap()
    ln1_w = nc.dram_tensor("ln1_w", [1, C], F32, kind="ExternalInput").ap()
    ln1_b = nc.dram_tensor("ln1_b", [1, C], F32, kind="ExternalInput").ap()
    ln2_w = nc.dram_tensor("ln2_w", [1, C], F32, kind="ExternalInput").ap()
    ln2_b = nc.dram_tensor("ln2_b", [1, C], F32, kind="ExternalInput").ap()
    out = nc.dram_tensor("out", [T, C], F16, kind="ExternalOutput").ap()

    # token-pair layout for DoubleRow: [p, pair, s, 2K]
    ekev_r = ekev_s.rearrange("(i s p) k -> p i s k", p=128, s=2)
    x_r = x_s.rearrange("(i p) c -> p i c", p=128)          # (128, 16, C)
    proj_w8_r = proj_w8.rearrange("w p j s c -> p w j s c")

    with tile.TileContext(nc) as tc, \
         nc.allow_low_precision(reason="fp16/fp8 pipeline validated vs fp32"):
      with tc.tile_pool(name="const", bufs=1) as constp, \
           tc.tile_pool(name="glob", bufs=1) as glob, \
           tc.tile_pool(name="dram", bufs=1, space="DRAM") as dram:
        qkvb = constp.tile([128, 18], F32, tag="qkvb")
        fc1b = constp.tile([128, 24], F32, tag="fc1b")
        qb_scaled = constp.tile([128, 6], F32, tag="qb_scaled")
        with tc.tile_wait_until(0.010):
            nc.sync.dma_start(qkvb[:], qkv_b.rearrange("(m p) -> p m", p=128))
            nc.sync.dma_start(fc1b[:], fc1_b.rearrange("(m p) -> p m", p=128))
        nc.vector.tensor_scalar_mul(qb_scaled[:], qkvb[:, 0:6], SCALE)

        def bcast_row(name, src_ap, width):
            row = constp.tile([1, width], F32, tag=f"{name}_row")
            nc.sync.dma_start(row[:], src_ap)
            bc = constp.tile([128, width], F32, tag=f"{name}_bc")
            nc.gpsimd.partition_broadcast(bc[:], row[:])
            return bc

        ln1w_bc = ln1b_bc = ln2w_bc = ln2b_bc = None
        projb_bc = fc2b_bc = None
        if not ln1_triv:
            ln1w_bc = bcast_row("ln1w", ln1_w[:], C)
            ln1b_bc = bcast_row("ln1b", ln1_b[:], C)
        if not ln2_triv:
            ln2w_bc = bcast_row("ln2w", ln2_w[:], C)
            ln2b_bc = bcast_row("ln2b", ln2_b[:], C)
        if not projb_zero:
            projb_bc = bcast_row("projb", proj_b[None, :], C)
        if not fc2b_zero:
            fc2b_bc = bcast_row("fc2b", fc2_b[None, :], C)
        assert kvb_zero, "nonzero k/v bias not supported by this kernel"

        # Globals that outlive the stage-1 pool: q^T, MLP weights, proj
        # weights, post-AR k^T / v (fp8 hi/lo).
        qT = glob.tile([128, CK, T], F16, tag="qT")
        w1 = glob.tile([128, 2, CP, 2, HID], F8, tag="w1")
        w2 = glob.tile([128, 2, HP, 2, C], F8, tag="w2")
        pw8 = glob.tile([128, 2, CP, 2, C], F8, tag="pw8")
        kT_sb = glob.tile([128, CK, K], F16, tag="kT")
        # v planes split by head parity (sub) with the other parity's
        # 64-column blocks zeroed, so a head pair's o^T accumulates into
        # one full PSUM tile (DoubleRow dst must start at partition 0).
        v8h = glob.tile([128, KC, 2, C], F8, tag="v8h")
        v8l = glob.tile([128, KC, 2, C], F8, tag="v8l")

        nc.gpsimd.memset(v8h[:], 0.0)
        nc.gpsimd.memset(v8l[:], 0.0)

        ar_in = dram.tile([128, 2, CK, K], F16)
        ar_out = dram.tile([128, 2, CK, K], F16)

        # ===== Stage 1: LN1, h1T, ekhT/evhT partials ======================
        # h1 is produced directly as scaled fp8 (x16) in chunk-paired
        # layout: bytes (chunk 2j, chunk 2j+1) adjacent, so the DMA
        # transpose moves them as fake fp16 and the q matmul reads a
        # strided fp8 pair view. The same tile, sliced by token parity,
        # feeds the ek/ev DoubleRow matmuls.
        with tc.tile_pool(name="s1", bufs=1) as s1p:
          h1Tp = s1p.tile([128, CP, T], F16, tag="h1Tp")
          qkvw8_sb = s1p.tile([128, 2, CP, 2, 3 * C], F8, tag="qkvw8")

          with tc.tile_pool(name="s1w", bufs=3) as wk, \
               tc.tile_pool(name="s1w2", bufs=2) as wk2, \
               tc.tile_pool(name="s1e", bufs=1) as s1e, \
               tc.tile_pool(name="psE", bufs=1, space="PSUM") as psE:
            ekacc = psE.tile([128, CK, K], F32, tag="ekacc")
            evacc = psE.tile([128, CK, K], F32, tag="evacc")
            for ip in range(NT // 2):
                # token-tile pair; x comes in quad DMAs (two pairs each)
                if ip % 2 == 0:
                    xt = wk.tile([128, 4, C], F16, tag="xt")
                    nc.sync.dma_start(xt[:], x_r[:, 2 * ip:2 * ip + 4, :])
                if ip in (0, 2):
                    # Wq (one plane per DMA) trickles in between the x/EkEv
                    # tile streams (needed right after pass 1 for qT);
                    # Wk/Wv stream later, during the qT/AllReduce window
                    wi = ip // 2
                    nc.sync.dma_start(qkvw8_sb[:, wi, :, :, 0:C],
                                      qkv_w8[wi, :, :, :, 0:C])
                ee = wk2.tile([128, 2, 2 * K], F8, tag="ee")
                nc.sync.dma_start(ee[:], ekev_r[:, ip, :, :])
                hp8 = wk2.tile([128, 2, CP, 128, 2], F8, tag="hp8")
                for s in range(2):
                    i = 2 * ip + s
                    xts = xt[:, (ip % 2) * 2 + s, :]
                    rstd, nmr = _ln_stats(nc, wk, xts, "ln1")
                    rstd16 = wk.tile([128, 1], F32, tag="rstd16")
                    nc.vector.tensor_scalar_mul(rstd16[:], rstd[:], S1H)
                    nmr16 = wk.tile([128, 1], F32, tag="nmr16")
                    nc.vector.tensor_scalar_mul(nmr16[:], nmr[:], S1H)
                    # LN apply straight to scaled fp8 in chunk-paired
                    # layout (on Pool: DVE is the busy engine in stage 1)
                    nc.gpsimd.tensor_scalar(
                        hp8[:, s].rearrange("p j c t -> p j t c"),
                        xts.rearrange("p (j t c) -> p j t c", j=CP, t=2),
                        rstd16[:], nmr16[:], op0=ALU.mult, op1=ALU.add)
                    if ln1w_bc is not None:
                        raise NotImplementedError(
                            "nontrivial ln1 unsupported by packed path")
                    nc.sync.dma_start_transpose(
                        h1Tp[:, :, i * 128:(i + 1) * 128],
                        hp8[:, s].bitcast(F16))
                st = (ip == 0)
                sp = (ip == NT // 2 - 1)
                for k in range(CK):
                    nc.tensor.matmul(ekacc[:, k, :],
                                     hp8[:, :, k // 2, :, k % 2],
                                     ee[:, :, 0:K],
                                     start=st, stop=sp, perf_mode=DR)
                    nc.tensor.matmul(evacc[:, k, :],
                                     hp8[:, :, k // 2, :, k % 2],
                                     ee[:, :, K:2 * K],
                                     start=st, stop=sp, perf_mode=DR)
            with tc.high_priority():
                ek_sb = s1e.tile([128, CK, K], F16, tag="ek_sb")
                nc.scalar.activation(ek_sb[:], ekacc[:], AF.Copy,
                                     scale=1.0 / (S1H * SEK))
                nc.sync.dma_start(ar_in[:, 0], ek_sb[:])
                ev_sb = s1e.tile([128, CK, K], F16, tag="ev_sb")
                nc.scalar.activation(ev_sb[:], evacc[:], AF.Copy,
                                     scale=1.0 / (S1H * SEK))
                nc.sync.dma_start(ar_in[:, 1], ev_sb[:])

          # One AllReduce for both halves: the collective cost is
          # overhead-dominated, so splitting it would double the cost.
          with tc.high_priority():
              nc.gpsimd.collective_compute(
                  "AllReduce", ALU.add,
                  replica_groups=[[0, 1, 2, 3], [4, 5, 6, 7]],
                  ins=[ar_in.opt()], outs=[ar_out.opt()])

          # Wk/Wv + proj weights stream in during the AllReduce + qT
          # window only (wait timestamps keep them out of stage 1's
          # DMA-bound window; kT needs them at ~60us, proj at ~72us)
          with tc.tile_wait_until(0.020):
              for wi in range(2):
                  for j in range(CP):
                      nc.sync.dma_start(qkvw8_sb[:, wi, j, :, C:3 * C],
                                        qkv_w8[wi, :, j, :, C:3 * C])
          with tc.tile_wait_until(0.028):
              nc.sync.dma_start(pw8[:], proj_w8_r)

          # ===== qT (overlaps the AllReduce) + post-AR k^T/v ==============
          with tc.tile_pool(name="psQ", bufs=2, space="PSUM") as psQ, \
               tc.tile_pool(name="par", bufs=1) as par, \
               tc.tile_pool(name="psP", bufs=2, space="PSUM") as psP:
            h1r = [h1Tp[:, j, g * 512:(g + 1) * 512].bitcast(F8).rearrange(
                       "p (n s) -> p s n", s=2)
                   for g in range(NG) for j in range(CP)]

            def qt_group(g):
                for m in range(CK):
                    qp = psQ.tile([128, 512], F32, tag="qp")
                    im = 0
                    for wi in range(2):
                        for j in range(CP):
                            nc.tensor.matmul(
                                qp[:],
                                qkvw8_sb[:, wi, j, :, m * 128:(m + 1) * 128],
                                h1r[g * CP + j],
                                start=(im == 0), stop=(im == 2 * CP - 1),
                                perf_mode=DR)
                            im += 1
                    nc.vector.tensor_scalar(
                        qT[:, m, g * 512:(g + 1) * 512], qp[:],
                        SCALE / (SH * SQW),
                        qb_scaled[:, m:m + 1], op0=ALU.mult, op1=ALU.add)

            # all q^T groups run during the AllReduce window (PE has
            # nothing else until ekh arrives)
            for g in range(NG):
                qt_group(g)
            # post-AR: k_projT = Wk^T ekhT
            with tc.high_priority():
                ekh = par.tile([128, CK, K], F16, tag="ekh")
                nc.sync.dma_start(ekh[:], ar_out[:, 0])
                evh = par.tile([128, CK, K], F16, tag="evh")
                nc.sync.dma_start(evh[:], ar_out[:, 1])
            ekh8 = par.tile([128, CK, K], F8, tag="ekh8")
            nc.gpsimd.tensor_scalar_mul(ekh8[:], ekh[:], SEH)
            for m in range(CK):
                kp = psP.tile([128, K], F32, tag="kp")
                im = 0
                for wi in range(2):
                    for j in range(CP):
                        nc.tensor.matmul(
                            kp[:],
                            qkvw8_sb[:, wi, j, :,
                                     C + m * 128:C + (m + 1) * 128],
                            ekh8[:, 2 * j:2 * j + 2, :],
                            start=(im == 0), stop=(im == 2 * CP - 1),
                            perf_mode=DR)
                        im += 1
                nc.vector.tensor_scalar_mul(kT_sb[:, m, :], kp[:],
                                            1.0 / (SEH * SQW))
            evh8 = par.tile([128, CK, K], F8, tag="evh8")
            nc.gpsimd.tensor_scalar_mul(evh8[:], evh[:], SEH)
            for kc in range(KC):
                for half in range(2):
                    c0 = half * 384
                    vp = psP.tile([128, 384], F32, tag="vp")
                    im = 0
                    for wi in range(2):
                        for j in range(CP):
                            nc.tensor.matmul(
                                vp[:],
                                evh8[:, 2 * j:2 * j + 2,
                                     kc * 128:(kc + 1) * 128],
                                qkvw8_sb[:, wi, j, :,
                                         2 * C + c0:2 * C + c0 + 384],
                                start=(im == 0), stop=(im == 2 * CP - 1),
                                perf_mode=DR)
                            im += 1
                    vpv = vp[:].rearrange("p (a t c) -> p a t c", t=2, c=64)
                    for sub in range(2):
                        vh = v8h[:, kc, sub, c0:c0 + 384].rearrange(
                            "p (a t c) -> p a t c", t=2, c=64)[:, :, sub, :]
                        nc.scalar.activation(vh, vpv[:, :, sub, :],
                                             AF.Copy, scale=SV / (SEH * SQW))
                        nc.vector.scalar_tensor_tensor(
                            v8l[:, kc, sub, c0:c0 + 384].rearrange(
                                "p (a t c) -> p a t c", t=2, c=64)[:, :, sub, :],
                            vpv[:, :, sub, :], SV / (SEH * SQW), vh,
                            op0=ALU.mult, op1=ALU.subtract)
        # s1 pool (h1T, h1T8, qkvw8) closes here

        # ===== Stage 2: attention + proj + LN2 + MLP per 512-token group ==
        from contextlib import ExitStack
        with ExitStack() as es:
            at = es.enter_context(tc.tile_pool(name="attn", bufs=3))
            at1 = es.enter_context(tc.tile_pool(name="attn1", bufs=2))
            at3 = es.enter_context(tc.tile_pool(name="attn3", bufs=3))
            pj = es.enter_context(tc.tile_pool(name="prj", bufs=1))
            pjr = es.enter_context(tc.tile_pool(name="prjr", bufs=2))
            pj1 = es.enter_context(tc.tile_pool(name="prj1", bufs=2))
            pj2 = es.enter_context(tc.tile_pool(name="prj2", bufs=3))
            pjx = es.enter_context(tc.tile_pool(name="prjx", bufs=4))
            ml = es.enter_context(tc.tile_pool(name="mlp", bufs=2))
            ml1 = es.enter_context(tc.tile_pool(name="mlp1", bufs=2))
            psL = es.enter_context(tc.tile_pool(name="psL", bufs=2, space="PSUM"))
            psCo = es.enter_context(tc.tile_pool(name="psCo", bufs=2, space="PSUM"))
            psC = es.enter_context(tc.tile_pool(name="psC", bufs=1, space="PSUM"))
            psF = es.enter_context(tc.tile_pool(name="psF", bufs=2, space="PSUM"))
            psD = es.enter_context(tc.tile_pool(name="psD", bufs=1, space="PSUM"))
            def attn_proj(g):
                if g == 0:
                    # fc1/fc2 weights stream in behind the g=0/g=1 attention
                    with tc.tile_wait_until(0.036):
                        for jj in range(2 * CP):
                            nc.sync.dma_start(
                                w1[:, jj // CP, jj % CP, :, :],
                                fc1_w8[jj // CP, :, jj % CP, :, :])
                if g == 1:
                    with tc.tile_wait_until(0.055):
                        for jj in range(4):
                            nc.sync.dma_start(
                                w2[:, jj // 2,
                                   6 * (jj % 2):6 * (jj % 2 + 1), :, :],
                                fc2_w8[jj // 2, :,
                                       6 * (jj % 2):6 * (jj % 2 + 1), :, :])
                # ---- attention for token group g ----
                oT8 = at1.tile([128, CK, 512], F8, tag="oT8")
                for ph in range(H // 2):
                    eTs = []
                    # combined denominators: partitions 0:64 hold head
                    # 2ph's denom, 64:128 hold head 2ph+1's
                    dnvP = at.tile([128, 512], F16, tag="dnvP")
                    for sub in range(2):
                        h = 2 * ph + sub
                        off = 64 * (h % 2)
                        ch = h // 2
                        eT = at.tile([128, KC, 512], F8, tag=f"eT{ph % 2}")
                        for kc in range(KC):
                            lg = psL.tile([128, 512], F32, tag="lg")
                            nc.tensor.matmul(
                                lg[:],
                                kT_sb[off:off + 64, ch,
                                      kc * 128:(kc + 1) * 128],
                                qT[off:off + 64, ch,
                                   g * 512:(g + 1) * 512],
                                start=True, stop=True,
                                tile_position=(off, 0))
                            nc.scalar.activation(eT[:, kc, :], lg[:], AF.Exp)
                        dnr = at3.tile([128, KC, 512], F16, tag="dnr")
                        nc.gpsimd.partition_all_reduce(
                            dnr[:], eT[:], channels=128,
                            reduce_op=bass.bass_isa.ReduceOp.add)
                        # kc-sum split across DVE/Pool to balance the
                        # attention-phase load
                        eng = nc.vector if sub == 0 else nc.gpsimd
                        eng.tensor_add(dnvP[off:off + 64, :],
                                       dnr[off:off + 64, 0, :],
                                       dnr[off:off + 64, 1, :])
                        eTs.append(eT)
                    rcbP = at3.tile([128, 512], F16, tag="rcbP")
                    nc.vector.reciprocal(rcbP[:], dnvP[:])
                    # o^T for the head pair in DoubleRow (kc is the pair
                    # axis, hi+lo v planes accumulate in PSUM): each head's
                    # 64 output rows land in its own half of one PSUM tile.
                    pa = psCo.tile([128, 512], F32, tag="oTps")
                    vs = ph * 128
                    for sub in range(2):
                        nc.tensor.matmul(pa[:],
                                         v8h[:, :, sub, vs:vs + 128],
                                         eTs[sub][:], start=(sub == 0),
                                         stop=False, perf_mode=DR)
                        nc.tensor.matmul(pa[:],
                                         v8l[:, :, sub, vs:vs + 128],
                                         eTs[sub][:], start=False,
                                         stop=(sub == 1), perf_mode=DR)
                    nc.vector.scalar_tensor_tensor(
                        oT8[:, ph, :], pa[:], SO / SV,
                        rcbP[:], op0=ALU.mult, op1=ALU.mult)

                # ---- proj + residual into x2g ----
                x2g = pjx.tile([128, 4, C], F16, tag="x2g")
                for ms in range(4):
                    r0 = g * 512 + ms * 128
                    if ms % 2 == 0:
                        xrp = pjr.tile([128, 2, C], F16, tag="xr")
                        nc.sync.dma_start(
                            xrp[:], x_r[:, 4 * g + ms:4 * g + ms + 2, :])
                    xr = xrp[:, ms % 2, :]
                    for cs in range(2):
                        pp = psC.tile([128, 384], F32, tag="pjtp")
                        im = 0
                        for wi in range(2):
                            for j in range(CP):
                                nc.tensor.matmul(
                                    pp[:],
                                    oT8[:, 2 * j:2 * j + 2,
                                        ms * 128:(ms + 1) * 128],
                                    pw8[:, wi, j, :,
                                        cs * 384:(cs + 1) * 384],
                                    start=(im == 0), stop=(im == 2 * CP - 1),
                                    perf_mode=DR)
                                im += 1
                        nc.vector.scalar_tensor_tensor(
                            x2g[:, ms, cs * 384:(cs + 1) * 384], pp[:],
                            1.0 / (SO * SPW),
                            xr[:, cs * 384:(cs + 1) * 384],
                            op0=ALU.mult, op1=ALU.add)
                    if projb_bc is not None:
                        nc.vector.tensor_add(x2g[:, ms, :], x2g[:, ms, :],
                                             projb_bc[:])
                return x2g

            def ln2_quant(g, x2g):
                # LN2 + transpose + fp8 quant for group g. rstd via DVE
                # rsqrt, batched over the 4 row tiles (no ACT Sqrt).
                # The LN apply writes h2 directly as fp8 with the two
                # chunks of each DoubleRow pair in adjacent bytes; the DMA
                # transpose then moves those 2-byte units as fake fp16, so
                # no separate fp16->fp8 conversion pass is needed.
                h2Tp = pj2.tile([128, CP, 512], F16, tag="h2Tp")
                NSUB = 3
                mvs = []
                var4 = pj.tile([128, 4], F32, tag="l2var4")
                for ms in range(4):
                    stats = pj.tile([128, NSUB, 6], F32, tag=f"l2s{ms}")
                    xv = x2g[:, ms, :].rearrange("p (j s) -> p j s", j=NSUB)
                    for j in range(NSUB):
                        nc.vector.bn_stats(stats[:, j, :], xv[:, j, :])
                    mv = pj.tile([128, 2], F32, tag=f"l2mv{ms}")
                    nc.vector.bn_aggr(mv[:], stats[:])
                    nc.vector.tensor_scalar_add(var4[:, ms:ms + 1],
                                                mv[:, 1:2], EPS)
                    mvs.append(mv)
                rstd4 = _rsqrt_dve(nc, pj, var4[:], 4, f"l2r{g % 2}")
                rstd4s = pj.tile([128, 4], F32, tag="l2rs")
                nc.vector.tensor_scalar_mul(rstd4s[:], rstd4, SX)
                for ms in range(4):
                    nmrs = pj.tile([128, 1], F32, tag=f"l2n{ms}")
                    nc.vector.scalar_tensor_tensor(
                        nmrs[:], mvs[ms][:, 0:1], -SX, rstd4[:, ms:ms + 1],
                        op0=ALU.mult, op1=ALU.mult)
                    h2p = pjr.tile([128, CP, 128, 2], F8, tag="h2p")
                    h2pw = h2p[:].rearrange("p j c s -> p j s c")
                    x2v = x2g[:, ms, :].rearrange("p (j s c) -> p j s c",
                                                  j=CP, s=2)
                    if ln2w_bc is None:
                        # apply emits SX-scaled fp8 in chunk-paired layout
                        # (on Pool: DVE is the attention-phase gate)
                        nc.gpsimd.tensor_scalar(
                            h2pw, x2v, rstd4s[:, ms:ms + 1],
                            nmrs[:], op0=ALU.mult, op1=ALU.add)
                    else:
                        h2 = pjr.tile([128, C], F16, tag="h2")
                        nc.vector.tensor_scalar(h2[:], x2g[:, ms, :],
                                                rstd4s[:, ms:ms + 1],
                                                nmrs[:],
                                                op0=ALU.mult, op1=ALU.add)
                        nc.vector.tensor_mul(h2[:], h2[:], ln2w_bc[:])
                        nc.vector.scalar_tensor_tensor(
                            h2pw,
                            ln2b_bc[:].rearrange("p (j s c) -> p j s c",
                                                 j=CP, s=2), SX,
                            h2[:].rearrange("p (j s c) -> p j s c",
                                            j=CP, s=2),
                            op0=ALU.mult, op1=ALU.add)
                    nc.sync.dma_start_transpose(
                        h2Tp[:, :, ms * 128:(ms + 1) * 128],
                        h2p[:].bitcast(F16))
                return h2Tp

            def mlp(g, x2g, h2Tp):
                # ---- MLP for group g: fp8 DoubleRow, W hi/lo, X single ----
                # rhs pairs come from the packed transpose: fp8 view with
                # the chunk-pair in adjacent bytes (s stride 1, n stride 2)
                h2r = [h2Tp[:, j, :].bitcast(F8).rearrange(
                    "p (n s) -> p s n", s=2) for j in range(CP)]
                gT = ml1.tile([128, HC, 512], F8, tag="gT")
                for hc in range(HC):
                    fp = psF.tile([128, 512], F32, tag="fp")
                    hs = hc * 128
                    im = 0
                    for wi in range(2):
                        for j in range(CP):
                            nc.tensor.matmul(
                                fp[:], w1[:, wi, j, :, hs:hs + 128],
                                h2r[j],
                                start=(im == 0), stop=(im == 2 * CP - 1),
                                perf_mode=DR)
                            im += 1
                    # gelu straight to fp8 (SG == 1)
                    nc.scalar.activation(gT[:, hc, :], fp[:], AF.Gelu,
                                         bias=fc1b[:, hc:hc + 1],
                                         scale=1.0 / (SX * SW))
                for ms in range(4):
                    r0 = g * 512 + ms * 128
                    oth = ml.tile([128, C], F16, tag="oth")
                    for cs in range(2):
                        op = psD.tile([128, 384], F32, tag="op")
                        im = 0
                        for wi in range(2):
                            for j in range(HP):
                                nc.tensor.matmul(
                                    op[:],
                                    gT[:, 2 * j:2 * j + 2,
                                       ms * 128:(ms + 1) * 128],
                                    w2[:, wi, j, :, cs * 384:(cs + 1) * 384],
                                    start=(im == 0), stop=(im == 2 * HP - 1),
                                    perf_mode=DR)
                                im += 1
                        nc.vector.scalar_tensor_tensor(
                            oth[:, cs * 384:(cs + 1) * 384], op[:], 1.0 / SW,
                            x2g[:, ms, cs * 384:(cs + 1) * 384],
                            op0=ALU.mult, op1=ALU.add)
                    if fc2b_bc is not None:
                        nc.vector.tensor_add(oth[:], oth[:], fc2b_bc[:])
                    nc.sync.dma_start(out[r0:r0 + 128, :], oth[:])

            # Issue order staggers mlp(g) right behind ln2(g) but behind the
            # NEXT attention group: each group's gelu block then runs
            # contiguously between exp blocks (priority heap keeps blocks
            # together -> few ACT table switches), and fc1/fc2 matmuls fill
            # PE bubbles during the ACT/DVE-bound attention phase.
            xs, ts = [None] * NG, [None] * NG
            xs[0] = attn_proj(0)
            xs[1] = attn_proj(1)
            ts[0] = ln2_quant(0, xs[0])
            xs[2] = attn_proj(2)
            ts[1] = ln2_quant(1, xs[1])
            xs[3] = attn_proj(3)
            ts[2] = ln2_quant(2, xs[2])
            ts[3] = ln2_quant(3, xs[3])
            for g in range(NG):
                mlp(g, xs[g], ts[g])

    nc.compile()
    return nc


def _hilo8(a, scale):
    hi = (a * scale).astype(ml_dtypes.float8_e4m3)
    lo = (a * scale - hi.astype(np.float32)).astype(ml_dtypes.float8_e4m3)
    return hi, lo


def _w_hilo_dr(w, scale, cin, cout):
    """[cin, cout] -> [2, 128, cin//256, 2, cout] hi/lo fp8 DR layout."""
    pairs = cin // 256
    h, lo = _hilo8(w, scale)
    f = np.stack([h, lo]).reshape(2, pairs, 2, 128, cout)
    return np.ascontiguousarray(f.transpose(0, 3, 1, 2, 4))


def kernel(**inputs):
    x = np.asarray(inputs["x"], dtype=np.float32)
    qkv_w = np.asarray(inputs["qkv_w"], dtype=np.float32)
    qkv_b = np.ascontiguousarray(np.asarray(inputs["qkv_b"], dtype=np.float32))
    Ek = np.asarray(inputs["Ek"], dtype=np.float32)
    Ev = np.asarray(inputs["Ev"], dtype=np.float32)
    proj_w = np.asarray(inputs["proj_w"], dtype=np.float32)
    proj_b = np.ascontiguousarray(np.asarray(inputs["proj_b"], dtype=np.float32))
    fc1_w = np.asarray(inputs["fc1_w"], dtype=np.float32)
    fc1_b = np.ascontiguousarray(np.asarray(inputs["fc1_b"], dtype=np.float32))
    fc2_w = np.asarray(inputs["fc2_w"], dtype=np.float32)
    fc2_b = np.ascontiguousarray(np.asarray(inputs["fc2_b"], dtype=np.float32))
    ln1_w = np.asarray(inputs["ln1_w"], dtype=np.float32)
    ln1_b = np.asarray(inputs["ln1_b"], dtype=np.float32)
    ln2_w = np.asarray(inputs["ln2_w"], dtype=np.float32)
    ln2_b = np.asarray(inputs["ln2_b"], dtype=np.float32)

    ln1_triv = bool(np.all(ln1_w == 1.0) and np.all(ln1_b == 0.0))
    ln2_triv = bool(np.all(ln2_w == 1.0) and np.all(ln2_b == 0.0))
    kvb_zero = bool(np.all(qkv_b[C:] == 0.0))
    projb_zero = bool(np.all(proj_b == 0.0))
    fc2b_zero = bool(np.all(fc2_b == 0.0))

    key = (ln1_triv, ln2_triv, kvb_zero, projb_zero, fc2b_zero)
    if key not in _CACHE:
        _CACHE[key] = build(*key)
    nc = _CACHE[key]

    qkv_w8 = _w_hilo_dr(qkv_w, SQW, C, 3 * C)
    proj_w8 = _w_hilo_dr(proj_w, SPW, C, C)
    fc1_w8 = _w_hilo_dr(fc1_w, SW, C, HID)
    fc2_w8 = _w_hilo_dr(fc2_w, SW, HID, C)
    ekev8 = np.concatenate(
        [(Ek * SEK).astype(ml_dtypes.float8_e4m3),
         (Ev * SEK).astype(ml_dtypes.float8_e4m3)], axis=1)

    xf = x.reshape(B * N, C).astype(np.float16)
    in_maps = []
    for c in range(NCORES):
        pos0 = (c % 4) * T
        in_maps.append({
            "x_s": np.ascontiguousarray(xf[c * T:(c + 1) * T]),
            "qkv_w8": qkv_w8,
            "qkv_b": qkv_b,
            "ekev_s": np.ascontiguousarray(ekev8[pos0:pos0 + T]),
            "proj_w8": proj_w8,
            "proj_b": proj_b,
            "fc1_w8": fc1_w8,
            "fc1_b": fc1_b,
            "fc2_w8": fc2_w8,
            "fc2_b": fc2_b,
            "ln1_w": np.ascontiguousarray(ln1_w.reshape(1, C)),
            "ln1_b": np.ascontiguousarray(ln1_b.reshape(1, C)),
            "ln2_w": np.ascontiguousarray(ln2_w.reshape(1, C)),
            "ln2_b": np.ascontiguousarray(ln2_b.reshape(1, C)),
        })

    import os
    trace = bool(os.environ.get("NN_BLOCK_TRACE"))
    res = run_bass_kernel_spmd(nc, in_maps, core_ids=list(range(NCORES)),
                               trace=trace)
    global LAST_RESULT
    LAST_RESULT = res
    outs = np.concatenate([res.results[c]["out"] for c in range(NCORES)],
                          axis=0)
    return outs.reshape(B, N, C).astype(np.float32)


LAST_RESULT = None


# revision 95
# speedup vs baseline: 1.0075x; 1.0075x over previous
"""Trainium2 Bass kernel for a Linformer transformer block (nn_Block).

Shapes (hardcoded): B=2, N=8192, C=768, H=12, D=64, K=256, HID=3072.
Sharding: 8 cores, data-parallel over tokens (2048 tokens/core, batch-major:
cores 0-3 hold batch 0, cores 4-7 batch 1).

Algorithm (per core, T=2048 tokens):
  - LN1 apply emits h1 directly as x16-scaled fp8 in "chunk-paired" layout
    (DoubleRow pair chunks in adjacent bytes), so one DMA transpose of the
    fake-fp16 view yields the q-matmul rhs and the same tile feeds the
    ek/ev DoubleRow matmuls -- no fp16 h1 and no conversion passes.
  - Linformer reorder: ekhT = h1^T Ek (C x K partials over local tokens),
    AllReduce over the batch group (all four q^T groups overlap it), then
    k_projT = Wk^T ekhT and v_proj = evhT^T Wv -- eliminates the two big
    [T,C]x[C,C] GEMMs.
  - All weight-stationary GEMMs (q/k/v proj, out-proj, fc1, fc2) run in
    fp8e4m3 DoubleRow mode (0.5 cyc/row, 256-deep contraction): weights are
    hi+lo fp8 planes quantized on the host (W ~= Wh + Wl, accurate to
    ~2^-8), activations are a single fp8 plane emitted directly in fp8 by
    the producing instruction.
  - Attention per 512-token group: logits fp16, exp -> eT in fp8 straight
    from the ACT engine; o^T for a head pair accumulates all four DoubleRow
    matmuls (hi/lo v planes, zero-padded by head parity) into one PSUM
    tile, so one reciprocal + one scale op normalizes both heads.
  - LN2 rstd computed on DVE (Quake rsqrt + 2 Newton steps) to keep the
    ACT table stream free of Sqrt; stage 2 issues all exp groups before
    all gelu groups so the ACT table loads only twice.
  - Engine balance: softmax denominator sums split DVE/Pool, LN applies on
    Pool, PSUM-reading drains on DVE; stage 1 is DMA-bound (x in quad
    DMAs, Ek|Ev merged fp8, out in fp16) with the fc weights streaming
    during the AllReduce and attention windows.
"""

import sys
sys.path.insert(0, "/opt/trn_rl_repo")

import numpy as np
import ml_dtypes

import concourse.bass as bass
import concourse.mybir as mybir
import concourse.tile as tile
from concourse import bacc
from concourse.bass_utils import run_bass_kernel_spmd

F32 = mybir.dt.float32
F16 = mybir.dt.float16
I32 = mybir.dt.int32
F8 = mybir.dt.float8e4
AF = mybir.ActivationFunctionType
ALU = mybir.AluOpType
DR = mybir.MatmulPerfMode.DoubleRow

B, N, C = 2, 8192, 768
H, K = 12, 256
D = C // H                 # 64
HID = 4 * C                # 3072
EPS = 1e-6
NCORES = 8
T = (B * N) // NCORES      # 2048 tokens per core
NT = T // 128              # 16 token tiles
NG = T // 512              # 4 token groups
CK = C // 128              # 6 chunks of C
CP = CK // 2               # 3 DoubleRow pairs of C
HC = HID // 128            # 24 hidden chunks
HP = HC // 2               # 12 DoubleRow pairs of HID
KC = K // 128              # 2 K chunks
SCALE = float(D) ** -0.5   # 0.125
SX = 16.0                  # fp8 scale for h2 (LN2 output)
SW = 64.0                  # fp8 scale for fc1/fc2 weights
SH = 16.0                  # fp8 scale for h1T (q input)
SQW = 1024.0               # fp8 scale for qkv_w planes
SPW = 1024.0               # fp8 scale for proj_w planes
SEH = 8.0                  # fp8 scale for ekh/evh (AllReduce output)
SEK = 1024.0               # fp8 scale for Ek/Ev host planes
S1H = 16.0                 # fp8 scale for h1 (stage-1 ek/ev matmuls)
SV = 16.0                  # fp8 scale for v_proj
SO = 16.0                  # fp8 scale for oT (attention output)
RSQRT_MAGIC = 0x5F3759DF

_CACHE = {}


def _ln_stats(nc, pool, xt, tag):
    """LayerNorm stats for a (128, C) fp32 tile -> (rstd, -mu*rstd) (128,1).
    Stage-1 variant: ACT Sqrt + DVE reciprocal (no exp/gelu nearby)."""
    NSUB = 3  # 768 = 3 x 256 (BN_STATS_FMAX=512, gcd=256)
    stats = pool.tile([128, NSUB, 6], F32, tag=f"{tag}_stats")
    xv = xt.rearrange("p (j s) -> p j s", j=NSUB)
    for j in range(NSUB):
        nc.vector.bn_stats(stats[:, j, :], xv[:, j, :])
    mv = pool.tile([128, 2], F32, tag=f"{tag}_mv")
    nc.vector.bn_aggr(mv[:], stats[:])
    var = pool.tile([128, 1], F32, tag=f"{tag}_var")
    nc.vector.tensor_scalar_add(var[:], mv[:, 1:2], EPS)
    std = pool.tile([128, 1], F32, tag=f"{tag}_std")
    nc.scalar.activation(std[:], var[:], AF.Sqrt)
    rstd = pool.tile([128, 1], F32, tag=f"{tag}_rstd")
    nc.vector.reciprocal(rstd[:], std[:])
    nmr = pool.tile([128, 1], F32, tag=f"{tag}_nmr")
    nc.vector.scalar_tensor_tensor(nmr[:], mv[:, 0:1], -1.0, rstd[:],
                                   op0=ALU.mult, op1=ALU.mult)
    return rstd, nmr


def _rsqrt_dve(nc, pool, var_ap, width, tag):
    """rstd = var ** -0.5 on DVE only (Quake seed + 2 Newton steps).
    var_ap: fp32 [128, width] AP. Keeps Sqrt off the ACT table stream."""
    yb = pool.tile([128, width], I32, tag=f"{tag}_yb")
    nc.vector.tensor_scalar(yb[:], var_ap.bitcast(I32), 1, -1,
                            op0=ALU.logical_shift_right, op1=ALU.bitwise_xor)
    nc.vector.tensor_scalar(yb[:], yb[:], RSQRT_MAGIC + 1, None, op0=ALU.add)
    y = yb[:].bitcast(F32)
    h = pool.tile([128, width], F32, tag=f"{tag}_h")
    for _ in range(2):
        nc.vector.tensor_mul(h[:], y, y)
        nc.vector.scalar_tensor_tensor(h[:], var_ap, -0.5, h[:],
                                       op0=ALU.mult, op1=ALU.mult)
        nc.vector.tensor_scalar_add(h[:], h[:], 1.5)
        nc.vector.tensor_mul(y, y, h[:])
    return y


def build(ln1_triv, ln2_triv, kvb_zero, projb_zero, fc2b_zero):
    nc = bacc.Bacc("TRN2", target_bir_lowering=False, debug=False,
                   enable_asserts=True, num_devices=NCORES)

    x_s = nc.dram_tensor("x_s", [T, C], F16, kind="ExternalInput").ap()
    # qkv weights: [hi/lo, 128, pair, 2, 3C] fp8, [p,j,s,m] = W[(2j+s)*128+p, m]
    qkv_w8 = nc.dram_tensor("qkv_w8", [2, 128, CP, 2, 3 * C], F8,
                            kind="ExternalInput").ap()
    qkv_b = nc.dram_tensor("qkv_b", [3 * C], F32, kind="ExternalInput").ap()
    # Ek/Ev merged, single-plane fp8 (scaled x1024 on host): halves their
    # DMA bytes and keeps 512B contiguous runs (no sub-512B DMA penalty).
    ekev_s = nc.dram_tensor("ekev_s", [T, 2 * K], F8,
                            kind="ExternalInput").ap()
    proj_w8 = nc.dram_tensor("proj_w8", [2, 128, CP, 2, C], F8,
                             kind="ExternalInput").ap()
    proj_b = nc.dram_tensor("proj_b", [C], F32, kind="ExternalInput").ap()
    # fc1 weights: [hi/lo, 128, pair, 2, HID] fp8, [p,j,s,m] = W1[(2j+s)*128+p, m]
    fc1_w8 = nc.dram_tensor("fc1_w8", [2, 128, CP, 2, HID], F8,
                            kind="ExternalInput").ap()
    fc1_b = nc.dram_tensor("fc1_b", [HID], F32, kind="ExternalInput").ap()
    # fc2 weights: [hi/lo, 128, pair, 2, C] fp8, [p,j,s,c] = W2[(2j+s)*128+p, c]
    fc2_w8 = nc.dram_tensor("fc2_w8", [2, 128, HP, 2, C], F8,
                            kind="ExternalInput").ap()
    fc2_b = nc.dram_tensor("fc2_b", [C], F32, kind="ExternalInput").ap()
    ln1_w = nc.dram_tensor("ln1_w", [1, C], F32, kind="ExternalInput").ap()
    ln1_b = nc.dram_tensor("ln1_b", [1, C], F32, kind="ExternalInput").ap()
    ln2_w = nc.dram_tensor("ln2_w", [1, C], F32, kind="ExternalInput").ap()
    ln2_b = nc.dram_tensor("ln2_b", [1, C], F32, kind="ExternalInput").ap()
    out = nc.dram_tensor("out", [T, C], F16, kind="ExternalOutput").ap()

    # token-pair layout for DoubleRow: [p, pair, s, 2K]
    ekev_r = ekev_s.rearrange("(i s p) k -> p i s k", p=128, s=2)
    x_r = x_s.rearrange("(i p) c -> p i c", p=128)          # (128, 16, C)
    proj_w8_r = proj_w8.rearrange("w p j s c -> p w j s c")

    with tile.TileContext(nc) as tc, \
         nc.allow_low_precision(reason="fp16/fp8 pipeline validated vs fp32"):
      with tc.tile_pool(name="const", bufs=1) as constp, \
           tc.tile_pool(name="glob", bufs=1) as glob, \
           tc.tile_pool(name="dram", bufs=1, space="DRAM") as dram:
        qkvb = constp.tile([128, 18], F32, tag="qkvb")
        fc1b = constp.tile([128, 24], F32, tag="fc1b")
        qb_scaled = constp.tile([128, 6], F32, tag="qb_scaled")
        with tc.tile_wait_until(0.010):
            nc.sync.dma_start(qkvb[:], qkv_b.rearrange("(m p) -> p m", p=128))
            nc.sync.dma_start(fc1b[:], fc1_b.rearrange("(m p) -> p m", p=128))
        nc.vector.tensor_scalar_mul(qb_scaled[:], qkvb[:, 0:6], SCALE)

        def bcast_row(name, src_ap, width):
            row = constp.tile([1, width], F32, tag=f"{name}_row")
            nc.sync.dma_start(row[:], src_ap)
            bc = constp.tile([128, width], F32, tag=f"{name}_bc")
            nc.gpsimd.partition_broadcast(bc[:], row[:])
            return bc

        ln1w_bc = ln1b_bc = ln2w_bc = ln2b_bc = None
        projb_bc = fc2b_bc = None
        if not ln1_triv:
            ln1w_bc = bcast_row("ln1w", ln1_w[:], C)
            ln1b_bc = bcast_row("ln1b", ln1_b[:], C)
        if not ln2_triv:
            ln2w_bc = bcast_row("ln2w", ln2_w[:], C)
            ln2b_bc = bcast_row("ln2b", ln2_b[:], C)
        if not projb_zero:
            projb_bc = bcast_row("projb", proj_b[None, :], C)
        if not fc2b_zero:
            fc2b_bc = bcast_row("fc2b", fc2_b[None, :], C)
        assert kvb_zero, "nonzero k/v bias not supported by this kernel"

        # Globals that outlive the stage-1 pool: q^T, MLP weights, proj
        # weights, post-AR k^T / v (fp8 hi/lo).
        qT = glob.tile([128, CK, T], F16, tag="qT")
        w1 = glob.tile([128, 2, CP, 2, HID], F8, tag="w1")
        w2 = glob.tile([128, 2, HP, 2, C], F8, tag="w2")
        pw8 = glob.tile([128, 2, CP, 2, C], F8, tag="pw8")
        kT_sb = glob.tile([128, CK, K], F16, tag="kT")
        # v planes split by head parity (sub) with the other parity's
        # 64-column blocks zeroed, so a head pair's o^T accumulates into
        # one full PSUM tile (DoubleRow dst must start at partition 0).
        v8h = glob.tile([128, KC, 2, C], F8, tag="v8h")
        v8l = glob.tile([128, KC, 2, C], F8, tag="v8l")

        nc.gpsimd.memset(v8h[:], 0.0)
        nc.gpsimd.memset(v8l[:], 0.0)

        ar_in = dram.tile([128, 2, CK, K], F16)
        ar_out = dram.tile([128, 2, CK, K], F16)

        # ===== Stage 1: LN1, h1T, ekhT/evhT partials ======================
        # h1 is produced directly as scaled fp8 (x16) in chunk-paired
        # layout: bytes (chunk 2j, chunk 2j+1) adjacent, so the DMA
        # transpose moves them as fake fp16 and the q matmul reads a
        # strided fp8 pair view. The same tile, sliced by token parity,
        # feeds the ek/ev DoubleRow matmuls.
        with tc.tile_pool(name="s1", bufs=1) as s1p:
          h1Tp = s1p.tile([128, CP, T], F16, tag="h1Tp")
          qkvw8_sb = s1p.tile([128, 2, CP, 2, 3 * C], F8, tag="qkvw8")

          with tc.tile_pool(name="s1w", bufs=3) as wk, \
               tc.tile_pool(name="s1w2", bufs=2) as wk2, \
               tc.tile_pool(name="s1e", bufs=1) as s1e, \
               tc.tile_pool(name="psE", bufs=1, space="PSUM") as psE:
            ekacc = psE.tile([128, CK, K], F32, tag="ekacc")
            evacc = psE.tile([128, CK, K], F32, tag="evacc")
            for ip in range(NT // 2):
                # token-tile pair; x comes in quad DMAs (two pairs each)
                if ip % 2 == 0:
                    xt = wk.tile([128, 4, C], F16, tag="xt")
                    nc.sync.dma_start(xt[:], x_r[:, 2 * ip:2 * ip + 4, :])
                if ip in (0, 2):
                    # Wq (one plane per DMA) trickles in between the x/EkEv
                    # tile streams (needed right after pass 1 for qT);
                    # Wk/Wv stream later, during the qT/AllReduce window
                    wi = ip // 2
                    nc.sync.dma_start(qkvw8_sb[:, wi, :, :, 0:C],
                                      qkv_w8[wi, :, :, :, 0:C])
                ee = wk2.tile([128, 2, 2 * K], F8, tag="ee")
                nc.sync.dma_start(ee[:], ekev_r[:, ip, :, :])
                hp8 = wk2.tile([128, 2, CP, 128, 2], F8, tag="hp8")
                for s in range(2):
                    i = 2 * ip + s
                    xts = xt[:, (ip % 2) * 2 + s, :]
                    rstd, nmr = _ln_stats(nc, wk, xts, "ln1")
                    rstd16 = wk.tile([128, 1], F32, tag="rstd16")
                    nc.vector.tensor_scalar_mul(rstd16[:], rstd[:], S1H)
                    nmr16 = wk.tile([128, 1], F32, tag="nmr16")
                    nc.vector.tensor_scalar_mul(nmr16[:], nmr[:], S1H)
                    # LN apply straight to scaled fp8 in chunk-paired
                    # layout (on Pool: DVE is the busy engine in stage 1)
                    nc.gpsimd.tensor_scalar(
                        hp8[:, s].rearrange("p j c t -> p j t c"),
                        xts.rearrange("p (j t c) -> p j t c", j=CP, t=2),
                        rstd16[:], nmr16[:], op0=ALU.mult, op1=ALU.add)
                    if ln1w_bc is not None:
                        raise NotImplementedError(
                            "nontrivial ln1 unsupported by packed path")
                    nc.sync.dma_start_transpose(
                        h1Tp[:, :, i * 128:(i + 1) * 128],
                        hp8[:, s].bitcast(F16))
                st = (ip == 0)
                sp = (ip == NT // 2 - 1)
                for k in range(CK):
                    nc.tensor.matmul(ekacc[:, k, :],
                                     hp8[:, :, k // 2, :, k % 2],
                                     ee[:, :, 0:K],
                                     start=st, stop=sp, perf_mode=DR)
                    nc.tensor.matmul(evacc[:, k, :],
                                     hp8[:, :, k // 2, :, k % 2],
                                     ee[:, :, K:2 * K],
                                     start=st, stop=sp, perf_mode=DR)
            with tc.high_priority():
                ek_sb = s1e.tile([128, CK, K], F16, tag="ek_sb")
                nc.scalar.activation(ek_sb[:], ekacc[:], AF.Copy,
                                     scale=1.0 / (S1H * SEK))
                nc.sync.dma_start(ar_in[:, 0], ek_sb[:])
                ev_sb = s1e.tile([128, CK, K], F16, tag="ev_sb")
                nc.scalar.activation(ev_sb[:], evacc[:], AF.Copy,
                                     scale=1.0 / (S1H * SEK))
                nc.sync.dma_start(ar_in[:, 1], ev_sb[:])

          # One AllReduce for both halves: the collective cost is
          # overhead-dominated, so splitting it would double the cost.
          with tc.high_priority():
              nc.gpsimd.collective_compute(
                  "AllReduce", ALU.add,
                  replica_groups=[[0, 1, 2, 3], [4, 5, 6, 7]],
                  ins=[ar_in.opt()], outs=[ar_out.opt()])

          # Wk/Wv + proj weights stream in during the AllReduce + qT
          # window only (wait timestamps keep them out of stage 1's
          # DMA-bound window; kT needs them at ~60us, proj at ~72us)
          with tc.tile_wait_until(0.030):
              for wi in range(2):
                  for j in range(CP):
                      nc.sync.dma_start(qkvw8_sb[:, wi, j, :, C:3 * C],
                                        qkv_w8[wi, :, j, :, C:3 * C])
          with tc.tile_wait_until(0.036):
              nc.sync.dma_start(pw8[:], proj_w8_r)

          # ===== qT (overlaps the AllReduce) + post-AR k^T/v ==============
          with tc.tile_pool(name="psQ", bufs=2, space="PSUM") as psQ, \
               tc.tile_pool(name="par", bufs=1) as par, \
               tc.tile_pool(name="psP", bufs=2, space="PSUM") as psP:
            h1r = [h1Tp[:, j, g * 512:(g + 1) * 512].bitcast(F8).rearrange(
                       "p (n s) -> p s n", s=2)
                   for g in range(NG) for j in range(CP)]

            def qt_group(g):
                for m in range(CK):
                    qp = psQ.tile([128, 512], F32, tag="qp")
                    im = 0
                    for wi in range(2):
                        for j in range(CP):
                            nc.tensor.matmul(
                                qp[:],
                                qkvw8_sb[:, wi, j, :, m * 128:(m + 1) * 128],
                                h1r[g * CP + j],
                                start=(im == 0), stop=(im == 2 * CP - 1),
                                perf_mode=DR)
                            im += 1
                    nc.vector.tensor_scalar(
                        qT[:, m, g * 512:(g + 1) * 512], qp[:],
                        SCALE / (SH * SQW),
                        qb_scaled[:, m:m + 1], op0=ALU.mult, op1=ALU.add)

            # all q^T groups run during the AllReduce window (PE has
            # nothing else until ekh arrives)
            for g in range(NG):
                qt_group(g)
            # post-AR: k_projT = Wk^T ekhT
            with tc.high_priority():
                ekh = par.tile([128, CK, K], F16, tag="ekh")
                nc.sync.dma_start(ekh[:], ar_out[:, 0])
                evh = par.tile([128, CK, K], F16, tag="evh")
                nc.sync.dma_start(evh[:], ar_out[:, 1])
            ekh8 = par.tile([128, CK, K], F8, tag="ekh8")
            nc.gpsimd.tensor_scalar_mul(ekh8[:], ekh[:], SEH)
            for m in range(CK):
                kp = psP.tile([128, K], F32, tag="kp")
                im = 0
                for wi in range(2):
                    for j in range(CP):
                        nc.tensor.matmul(
                            kp[:],
                            qkvw8_sb[:, wi, j, :,
                                     C + m * 128:C + (m + 1) * 128],
                            ekh8[:, 2 * j:2 * j + 2, :],
                            start=(im == 0), stop=(im == 2 * CP - 1),
                            perf_mode=DR)
                        im += 1
                nc.vector.tensor_scalar_mul(kT_sb[:, m, :], kp[:],
                                            1.0 / (SEH * SQW))
            evh8 = par.tile([128, CK, K], F8, tag="evh8")
            nc.gpsimd.tensor_scalar_mul(evh8[:], evh[:], SEH)
            for kc in range(KC):
                for half in range(2):
                    c0 = half * 384
                    vp = psP.tile([128, 384], F32, tag="vp")
                    im = 0
                    for wi in range(2):
                        for j in range(CP):
                            nc.tensor.matmul(
                                vp[:],
                                evh8[:, 2 * j:2 * j + 2,
                                     kc * 128:(kc + 1) * 128],
                                qkvw8_sb[:, wi, j, :,
                                         2 * C + c0:2 * C + c0 + 384],
                                start=(im == 0), stop=(im == 2 * CP - 1),
                                perf_mode=DR)
                            im += 1
                    vpv = vp[:].rearrange("p (a t c) -> p a t c", t=2, c=64)
                    for sub in range(2):
                        vh = v8h[:, kc, sub, c0:c0 + 384].rearrange(
                            "p (a t c) -> p a t c", t=2, c=64)[:, :, sub, :]
                        nc.scalar.activation(vh, vpv[:, :, sub, :],
                                             AF.Copy, scale=SV / (SEH * SQW))
                        nc.vector.scalar_tensor_tensor(
                            v8l[:, kc, sub, c0:c0 + 384].rearrange(
                                "p (a t c) -> p a t c", t=2, c=64)[:, :, sub, :],
                            vpv[:, :, sub, :], SV / (SEH * SQW), vh,
                            op0=ALU.mult, op1=ALU.subtract)
        # s1 pool (h1T, h1T8, qkvw8) closes here

        # ===== Stage 2: attention + proj + LN2 + MLP per 512-token group ==
        from contextlib import ExitStack
        with ExitStack() as es:
            at = es.enter_context(tc.tile_pool(name="attn", bufs=3))
            at1 = es.enter_context(tc.tile_pool(name="attn1", bufs=2))
            at3 = es.enter_context(tc.tile_pool(name="attn3", bufs=3))
            pj = es.enter_context(tc.tile_pool(name="prj", bufs=1))
            pjr = es.enter_context(tc.tile_pool(name="prjr", bufs=2))
            pj1 = es.enter_context(tc.tile_pool(name="prj1", bufs=2))
            pj2 = es.enter_context(tc.tile_pool(name="prj2", bufs=3))
            pjx = es.enter_context(tc.tile_pool(name="prjx", bufs=4))
            ml = es.enter_context(tc.tile_pool(name="mlp", bufs=2))
            ml1 = es.enter_context(tc.tile_pool(name="mlp1", bufs=2))
            psL = es.enter_context(tc.tile_pool(name="psL", bufs=2, space="PSUM"))
            psCo = es.enter_context(tc.tile_pool(name="psCo", bufs=2, space="PSUM"))
            psC = es.enter_context(tc.tile_pool(name="psC", bufs=1, space="PSUM"))
            psF = es.enter_context(tc.tile_pool(name="psF", bufs=2, space="PSUM"))
            psD = es.enter_context(tc.tile_pool(name="psD", bufs=1, space="PSUM"))
            def attn_proj(g):
                if g == 0:
                    # fc1/fc2 weights stream in behind the g=0/g=1 attention
                    with tc.tile_wait_until(0.042):
                        for jj in range(2 * CP):
                            nc.sync.dma_start(
                                w1[:, jj // CP, jj % CP, :, :],
                                fc1_w8[jj // CP, :, jj % CP, :, :])
                if g == 1:
                    with tc.tile_wait_until(0.060):
                        for jj in range(4):
                            nc.sync.dma_start(
                                w2[:, jj // 2,
                                   6 * (jj % 2):6 * (jj % 2 + 1), :, :],
                                fc2_w8[jj // 2, :,
                                       6 * (jj % 2):6 * (jj % 2 + 1), :, :])
                # ---- attention for token group g ----
                oT8 = at1.tile([128, CK, 512], F8, tag="oT8")
                for ph in range(H // 2):
                    eTs = []
                    # combined denominators: partitions 0:64 hold head
                    # 2ph's denom, 64:128 hold head 2ph+1's
                    dnvP = at.tile([128, 512], F16, tag="dnvP")
                    for sub in range(2):
                        h = 2 * ph + sub
                        off = 64 * (h % 2)
                        ch = h // 2
                        eT = at.tile([128, KC, 512], F8, tag=f"eT{ph % 2}")
                        for kc in range(KC):
                            lg = psL.tile([128, 512], F32, tag="lg")
                            nc.tensor.matmul(
                                lg[:],
                                kT_sb[off:off + 64, ch,
                                      kc * 128:(kc + 1) * 128],
                                qT[off:off + 64, ch,
                                   g * 512:(g + 1) * 512],
                                start=True, stop=True,
                                tile_position=(off, 0))
                            nc.scalar.activation(eT[:, kc, :], lg[:], AF.Exp)
                        dnr = at3.tile([128, KC, 512], F16, tag="dnr")
                        nc.gpsimd.partition_all_reduce(
                            dnr[:], eT[:], channels=128,
                            reduce_op=bass.bass_isa.ReduceOp.add)
                        # kc-sum split across DVE/Pool to balance the
                        # attention-phase load
                        eng = nc.vector if sub == 0 else nc.gpsimd
                        eng.tensor_add(dnvP[off:off + 64, :],
                                       dnr[off:off + 64, 0, :],
                                       dnr[off:off + 64, 1, :])
                        eTs.append(eT)
                    rcbP = at3.tile([128, 512], F16, tag="rcbP")
                    nc.vector.reciprocal(rcbP[:], dnvP[:])
                    # o^T for the head pair in DoubleRow (kc is the pair
                    # axis, hi+lo v planes accumulate in PSUM): each head's
                    # 64 output rows land in its own half of one PSUM tile.
                    pa = psCo.tile([128, 512], F32, tag="oTps")
                    vs = ph * 128
                    for sub in range(2):
                        nc.tensor.matmul(pa[:],
                                         v8h[:, :, sub, vs:vs + 128],
                                         eTs[sub][:], start=(sub == 0),
                                         stop=False, perf_mode=DR)
                        nc.tensor.matmul(pa[:],
                                         v8l[:, :, sub, vs:vs + 128],
                                         eTs[sub][:], start=False,
                                         stop=(sub == 1), perf_mode=DR)
                    nc.vector.scalar_tensor_tensor(
                        oT8[:, ph, :], pa[:], SO / SV,
                        rcbP[:], op0=ALU.mult, op1=ALU.mult)

                # ---- proj + residual into x2g ----
                x2g = pjx.tile([128, 4, C], F16, tag="x2g")
                for ms in range(4):
                    r0 = g * 512 + ms * 128
                    if ms % 2 == 0:
                        xrp = pjr.tile([128, 2, C], F16, tag="xr")
                        nc.sync.dma_start(
                            xrp[:], x_r[:, 4 * g + ms:4 * g + ms + 2, :])
                    xr = xrp[:, ms % 2, :]
                    for cs in range(2):
                        pp = psC.tile([128, 384], F32, tag="pjtp")
                        im = 0
                        for wi in range(2):
                            for j in range(CP):
                                nc.tensor.matmul(
                                    pp[:],
                                    oT8[:, 2 * j:2 * j + 2,
                                        ms * 128:(ms + 1) * 128],
                                    pw8[:, wi, j, :,
                                        cs * 384:(cs + 1) * 384],
                                    start=(im == 0), stop=(im == 2 * CP - 1),
                                    perf_mode=DR)
                                im += 1
                        nc.vector.scalar_tensor_tensor(
                            x2g[:, ms, cs * 384:(cs + 1) * 384], pp[:],
                            1.0 / (SO * SPW),
                            xr[:, cs * 384:(cs + 1) * 384],
                            op0=ALU.mult, op1=ALU.add)
                    if projb_bc is not None:
                        nc.vector.tensor_add(x2g[:, ms, :], x2g[:, ms, :],
                                             projb_bc[:])
                return x2g

            def ln2_quant(g, x2g):
                # LN2 + transpose + fp8 quant for group g. rstd via DVE
                # rsqrt, batched over the 4 row tiles (no ACT Sqrt).
                # The LN apply writes h2 directly as fp8 with the two
                # chunks of each DoubleRow pair in adjacent bytes; the DMA
                # transpose then moves those 2-byte units as fake fp16, so
                # no separate fp16->fp8 conversion pass is needed.
                h2Tp = pj2.tile([128, CP, 512], F16, tag="h2Tp")
                NSUB = 3
                mvs = []
                var4 = pj.tile([128, 4], F32, tag="l2var4")
                for ms in range(4):
                    stats = pj.tile([128, NSUB, 6], F32, tag=f"l2s{ms}")
                    xv = x2g[:, ms, :].rearrange("p (j s) -> p j s", j=NSUB)
                    for j in range(NSUB):
                        nc.vector.bn_stats(stats[:, j, :], xv[:, j, :])
                    mv = pj.tile([128, 2], F32, tag=f"l2mv{ms}")
                    nc.vector.bn_aggr(mv[:], stats[:])
                    nc.vector.tensor_scalar_add(var4[:, ms:ms + 1],
                                                mv[:, 1:2], EPS)
                    mvs.append(mv)
                rstd4 = _rsqrt_dve(nc, pj, var4[:], 4, f"l2r{g % 2}")
                rstd4s = pj.tile([128, 4], F32, tag="l2rs")
                nc.vector.tensor_scalar_mul(rstd4s[:], rstd4, SX)
                for ms in range(4):
                    nmrs = pj.tile([128, 1], F32, tag=f"l2n{ms}")
                    nc.vector.scalar_tensor_tensor(
                        nmrs[:], mvs[ms][:, 0:1], -SX, rstd4[:, ms:ms + 1],
                        op0=ALU.mult, op1=ALU.mult)
                    h2p = pjr.tile([128, CP, 128, 2], F8, tag="h2p")
                    h2pw = h2p[:].rearrange("p j c s -> p j s c")
                    x2v = x2g[:, ms, :].rearrange("p (j s c) -> p j s c",
                                                  j=CP, s=2)
                    if ln2w_bc is None:
                        # apply emits SX-scaled fp8 in chunk-paired layout
                        # (on Pool: DVE is the attention-phase gate)
                        nc.gpsimd.tensor_scalar(
                            h2pw, x2v, rstd4s[:, ms:ms + 1],
                            nmrs[:], op0=ALU.mult, op1=ALU.add)
                    else:
                        h2 = pjr.tile([128, C], F16, tag="h2")
                        nc.vector.tensor_scalar(h2[:], x2g[:, ms, :],
                                                rstd4s[:, ms:ms + 1],
                                                nmrs[:],
                                                op0=ALU.mult, op1=ALU.add)
                        nc.vector.tensor_mul(h2[:], h2[:], ln2w_bc[:])
                        nc.vector.scalar_tensor_tensor(
                            h2pw,
                            ln2b_bc[:].rearrange("p (j s c) -> p j s c",
                                                 j=CP, s=2), SX,
                            h2[:].rearrange("p (j s c) -> p j s c",
                                            j=CP, s=2),
                            op0=ALU.mult, op1=ALU.add)
                    nc.sync.dma_start_transpose(
                        h2Tp[:, :, ms * 128:(ms + 1) * 128],
                        h2p[:].bitcast(F16))
                return h2Tp

            def mlp(g, x2g, h2Tp):
                # ---- MLP for group g: fp8 DoubleRow, W hi/lo, X single ----
                # rhs pairs come from the packed transpose: fp8 view with
                # the chunk-pair in adjacent bytes (s stride 1, n stride 2)
                h2r = [h2Tp[:, j, :].bitcast(F8).rearrange(
                    "p (n s) -> p s n", s=2) for j in range(CP)]
                gT = ml1.tile([128, HC, 512], F8, tag="gT")
                for hc in range(HC):
                    fp = psF.tile([128, 512], F32, tag="fp")
                    hs = hc * 128
                    im = 0
                    for wi in range(2):
                        for j in range(CP):
                            nc.tensor.matmul(
                                fp[:], w1[:, wi, j, :, hs:hs + 128],
                                h2r[j],
                                start=(im == 0), stop=(im == 2 * CP - 1),
                                perf_mode=DR)
                            im += 1
                    # gelu straight to fp8 (SG == 1)
                    nc.scalar.activation(gT[:, hc, :], fp[:], AF.Gelu,
                                         bias=fc1b[:, hc:hc + 1],
                                         scale=1.0 / (SX * SW))
                for ms in range(4):
                    r0 = g * 512 + ms * 128
                    oth = ml.tile([128, C], F16, tag="oth")
                    for cs in range(2):
                        op = psD.tile([128, 384], F32, tag="op")
                        im = 0
                        for wi in range(2):
                            for j in range(HP):
                                nc.tensor.matmul(
                                    op[:],
                                    gT[:, 2 * j:2 * j + 2,
                                       ms * 128:(ms + 1) * 128],
                                    w2[:, wi, j, :, cs * 384:(cs + 1) * 384],
                                    start=(im == 0), stop=(im == 2 * HP - 1),
                                    perf_mode=DR)
                                im += 1
                        nc.vector.scalar_tensor_tensor(
                            oth[:, cs * 384:(cs + 1) * 384], op[:], 1.0 / SW,
                            x2g[:, ms, cs * 384:(cs + 1) * 384],
                            op0=ALU.mult, op1=ALU.add)
                    if fc2b_bc is not None:
                        nc.vector.tensor_add(oth[:], oth[:], fc2b_bc[:])
                    nc.sync.dma_start(out[r0:r0 + 128, :], oth[:])

            # Issue order staggers mlp(g) right behind ln2(g) but behind the
            # NEXT attention group: each group's gelu block then runs
            # contiguously between exp blocks (priority heap keeps blocks
            # together -> few ACT table switches), and fc1/fc2 matmuls fill
            # PE bubbles during the ACT/DVE-bound attention phase.
            xs, ts = [None] * NG, [None] * NG
            xs[0] = attn_proj(0)
            xs[1] = attn_proj(1)
            ts[0] = ln2_quant(0, xs[0])
            xs[2] = attn_proj(2)
            ts[1] = ln2_quant(1, xs[1])
            xs[3] = attn_proj(3)
            ts[2] = ln2_quant(2, xs[2])
            ts[3] = ln2_quant(3, xs[3])
            for g in range(NG):
                mlp(g, xs[g], ts[g])

    nc.compile()
    return nc


def _hilo8(a, scale):
    hi = (a * scale).astype(ml_dtypes.float8_e4m3)
    lo = (a * scale - hi.astype(np.float32)).astype(ml_dtypes.float8_e4m3)
    return hi, lo


def _w_hilo_dr(w, scale, cin, cout):
    """[cin, cout] -> [2, 128, cin//256, 2, cout] hi/lo fp8 DR layout."""
    pairs = cin // 256
    h, lo = _hilo8(w, scale)
    f = np.stack([h, lo]).reshape(2, pairs, 2, 128, cout)
    return np.ascontiguousarray(f.transpose(0, 3, 1, 2, 4))


def kernel(**inputs):
    x = np.asarray(inputs["x"], dtype=np.float32)
    qkv_w = np.asarray(inputs["qkv_w"], dtype=np.float32)
    qkv_b = np.ascontiguousarray(np.asarray(inputs["qkv_b"], dtype=np.float32))
    Ek = np.asarray(inputs["Ek"], dtype=np.float32)
    Ev = np.asarray(inputs["Ev"], dtype=np.float32)
    proj_w = np.asarray(inputs["proj_w"], dtype=np.float32)
    proj_b = np.ascontiguousarray(np.asarray(inputs["proj_b"], dtype=np.float32))
    fc1_w = np.asarray(inputs["fc1_w"], dtype=np.float32)
    fc1_b = np.ascontiguousarray(np.asarray(inputs["fc1_b"], dtype=np.float32))
    fc2_w = np.asarray(inputs["fc2_w"], dtype=np.float32)
    fc2_b = np.ascontiguousarray(np.asarray(inputs["fc2_b"], dtype=np.float32))
    ln1_w = np.asarray(inputs["ln1_w"], dtype=np.float32)
    ln1_b = np.asarray(inputs["ln1_b"], dtype=np.float32)
    ln2_w = np.asarray(inputs["ln2_w"], dtype=np.float32)
    ln2_b = np.asarray(inputs["ln2_b"], dtype=np.float32)

    ln1_triv = bool(np.all(ln1_w == 1.0) and np.all(ln1_b == 0.0))
    ln2_triv = bool(np.all(ln2_w == 1.0) and np.all(ln2_b == 0.0))
    kvb_zero = bool(np.all(qkv_b[C:] == 0.0))
    projb_zero = bool(np.all(proj_b == 0.0))
    fc2b_zero = bool(np.all(fc2_b == 0.0))

    key = (ln1_triv, ln2_triv, kvb_zero, projb_zero, fc2b_zero)
    if key not in _CACHE:
        _CACHE[key] = build(*key)
    nc = _CACHE[key]

    qkv_w8 = _w_hilo_dr(qkv_w, SQW, C, 3 * C)
    proj_w8 = _w_hilo_dr(proj_w, SPW, C, C)
    fc1_w8 = _w_hilo_dr(fc1_w, SW, C, HID)
    fc2_w8 = _w_hilo_dr(fc2_w, SW, HID, C)
    ekev8 = np.concatenate(
        [(Ek * SEK).astype(ml_dtypes.float8_e4m3),
         (Ev * SEK).astype(ml_dtypes.float8_e4m3)], axis=1)

    xf = x.reshape(B * N, C).astype(np.float16)
    in_maps = []
    for c in range(NCORES):
        pos0 = (c % 4) * T
        in_maps.append({
            "x_s": np.ascontiguousarray(xf[c * T:(c + 1) * T]),
            "qkv_w8": qkv_w8,
            "qkv_b": qkv_b,
            "ekev_s": np.ascontiguousarray(ekev8[pos0:pos0 + T]),
            "proj_w8": proj_w8,
            "proj_b": proj_b,
            "fc1_w8": fc1_w8,
            "fc1_b": fc1_b,
            "fc2_w8": fc2_w8,
            "fc2_b": fc2_b,
            "ln1_w": np.ascontiguousarray(ln1_w.reshape(1, C)),
            "ln1_b": np.ascontiguousarray(ln1_b.reshape(1, C)),
            "ln2_w": np.ascontiguousarray(ln2_w.reshape(1, C)),
            "ln2_b": np.ascontiguousarray(ln2_b.reshape(1, C)),
        })

    import os
    trace = bool(os.environ.get("NN_BLOCK_TRACE"))
    res = run_bass_kernel_spmd(nc, in_maps, core_ids=list(range(NCORES)),
                               trace=trace)
    global LAST_RESULT
    LAST_RESULT = res
    outs = np.concatenate([res.results[c]["out"] for c in range(NCORES)],
                          axis=0)
    return outs.reshape(B, N, C).astype(np.float32)


LAST_RESULT = None
